# revision 23
# baseline (speedup 1.0000x reference)
"""DirGATConv on 8 Trainium2 NeuronCores (Bass/Tile), v2.

Strategy (node/data parallel, no collectives):
  - Each core owns 6250 destination nodes, permuted into NBIN blocks of <=128
    by bin packing so every (block, direction, src-bank) has at most CB*128
    non-self-loop edges.
  - Phase A (replicated on every core): h = x @ W_d for all nodes plus the
    per-node attention projections es/ed = x @ (W_d a_*), written to two DRAM
    gather tables per direction (fp16 rows: 256 h | 4 es | 124 pad = 768 B;
    row count per bank <= 32767 because dma_gather indices are int16).  A
    bin-permuted local table per direction holds (h | es | ed) for the core's
    own destinations (544 B rows, read linearly in Phase B).
  - Phase B per (block, direction): dma_gather the source rows (one gather
    per src-bank), then with host-shipped 0/1 fp16 masks M [e,d] / MT [d,e]:
      ed_bc  = MT^T @ ed_tile                    (per-edge dst projection)
      p      = exp(lrelu(es + ed_bc) - ln 64)    (scalar engine; -ln64 keeps
                                                  h*p inside fp16 range)
      rows  *= p (per-head broadcast multiply), then one matmul per chunk
      num    = M^T @ rows, den = M^T @ p         (same stationary mask)
      out_d  = (num + p_self*h_loc) / (2*(den + p_self))
    Softmax normalization is exact because num and den are linear in p and
    any per-edge common factor (the -ln64 bias) cancels in num/den.
  - Host work is graph-structure-only (bin packing, gather indices, masks,
    layout transposes) plus standard weight fusion (W @ a projections).
"""

import numpy as np

import concourse.bacc as bacc
import concourse.mybir as mybir
import concourse.tile as tile
from concourse.bass_utils import run_bass_kernel_spmd
from concourse import library_config

# problem constants
N, E, DIN, H, C = 50000, 400000, 256, 4, 64
HC = H * C
ALPHA, SLOPE = 0.5, 0.2

# distribution constants
NCORES = 8
NPC = N // NCORES              # 6250 destinations per core
B0REAL = 24960                 # real nodes in bank 0 (nodes 0..24959)
BKROWS = 25088                 # rows per table bank (includes zero pad rows)
PADIDX = BKROWS - 1            # gather index for empty edge slots (zero row)
NT = 392                       # main node tiles (2 banks x 196)
NBIN = 50                      # destination blocks per core
CB = 4                         # gather chunks per (block, src-bank)
CPB = 2 * CB                   # chunks per block
NLOC = NBIN * 128
TW = 384                       # table row width (fp16) = 768 B
LW = 272                       # local row width (fp16) = 544 B
LNB = float(np.log(64.0))      # exp bias, cancels in num/den
F16 = mybir.dt.float16
F32 = mybir.dt.float32
I16 = mybir.dt.int16
AF = mybir.ActivationFunctionType
OP = mybir.AluOpType


def build_kernel(nbin=NBIN, cb=CB, debug=False):
    cpb = 2 * cb
    nc = bacc.Bacc("TRN2", num_swdge_queues=4)
    if debug:
        dbg_srcg = nc.dram_tensor("dbg_srcg", [128, cpb * TW], F16, kind="ExternalOutput")
        dbg_ped = nc.dram_tensor("dbg_ped", [128, cpb * 4], F32, kind="ExternalOutput")
        dbg_p = nc.dram_tensor("dbg_p", [128, cpb * 4], F16, kind="ExternalOutput")
        dbg_pnd = nc.dram_tensor("dbg_pnd", [128, HC + 4], F32, kind="ExternalOutput")
        dbg_lc = nc.dram_tensor("dbg_lc", [128, LW], F16, kind="ExternalOutput")
        dbg_srcg2 = nc.dram_tensor("dbg_srcg2", [128, cpb * TW], F16, kind="ExternalOutput")

    xTb = nc.dram_tensor("xTb", [2, 128, NT * 128], F16, kind="ExternalInput")
    xTl = nc.dram_tensor("xTl", [2, 128, nbin * 128], F16, kind="ExternalInput")
    Wsb = nc.dram_tensor("Wsb", [2, 2, 128, HC + 8], F16, kind="ExternalInput")
    b_in = nc.dram_tensor("b_in", [1, HC], F32, kind="ExternalInput")
    gidx = nc.dram_tensor("gidx", [2, nbin, 128, 16 * cb], I16, kind="ExternalInput")
    msk = nc.dram_tensor("msk", [2, nbin, 2, 128, cpb * 128], F16, kind="ExternalInput")
    out = nc.dram_tensor("out", [nbin * 128, HC], F32, kind="ExternalOutput")

    with tile.TileContext(nc) as tc:
        with (
            tc.tile_pool(name="dram", bufs=1, space="DRAM") as dpool,
            tc.tile_pool(name="const", bufs=1) as cpool,
        ):
            nc.gpsimd.load_library(library_config.mlp)

            tabs = [
                [dpool.tile([BKROWS, TW], F16, tag=f"tab{d}{k}", name=f"tab{d}{k}")
                 for k in range(2)]
                for d in range(2)
            ]
            locs = [dpool.tile([nbin * 128, LW], F16, tag=f"loc{d}", name=f"loc{d}")
                    for d in range(2)]

            # weights: w_sb[d] [128 din, 2 k, 264 = hc|es|ed]
            w_sb = [cpool.tile([128, 2, HC + 8], F16, tag=f"w{d}", name=f"w{d}")
                    for d in range(2)]
            for d in range(2):
                nc.sync.dma_start(
                    w_sb[d][:], Wsb[d].rearrange("k p c -> p k c"))
            bias_bc = cpool.tile([128, HC], F32)
            nc.sync.dma_start(bias_bc[:], b_in[:].to_broadcast((128, HC)))
            lnb_t = cpool.tile([128, 1], F32)
            nc.vector.memset(lnb_t[:], -LNB)

            # ---------------- Phase A ----------------
            with (
                tc.tile_pool(name="pAx", bufs=8) as pax,
                tc.tile_pool(name="pAs", bufs=6) as pas,
                tc.tile_pool(name="psA", bufs=4, space="PSUM") as psa,
            ):
                st = [None, None]

                def node_tile(xt_k, j, wide):
                    """One 128-node tile: xt_k [128, 2, 128]; write into
                    st[d][:, j, :] (wide=TW) or st[d] [128, LW] (wide=LW)."""
                    ph0 = psa.tile([128, HC + 8], F32, tag="ph0")
                    ph1 = psa.tile([128, HC + 8], F32, tag="ph1")
                    ph = [ph0, ph1]
                    for k in range(2):
                        for d in range(2):
                            nc.tensor.matmul(
                                ph[d][:], xt_k[:, k, :], w_sb[d][:, k, :],
                                start=(k == 0), stop=(k == 1))
                    if wide == TW:
                        nc.vector.tensor_copy(st[0][:, j, 0:HC + 8], ph[0][:])
                        nc.scalar.activation(st[1][:, j, 0:HC + 8], ph[1][:], AF.Copy)
                    else:
                        nc.vector.tensor_copy(st[0][:, 0:HC + 8], ph[0][:])
                        nc.scalar.activation(st[1][:, 0:HC + 8], ph[1][:], AF.Copy)

                # main tiles: 4-tile batches (bank boundary at tile 196 = 49*4)
                for it in range(NT // 4):
                    xt = pax.tile([128, 4, 2, 128], F16, tag="xt")
                    for k in range(2):
                        nc.sync.dma_start(
                            xt[:, :, k, :],
                            xTb[k, :, it * 512:(it + 1) * 512].rearrange(
                                "p (t c) -> p t c", c=128))
                    for d in range(2):
                        st[d] = pas.tile([128, 4, TW], F16, tag=f"st{d}", name=f"st{d}")
                    for t in range(4):
                        node_tile(xt[:, t, :, :], t, TW)
                    t0 = it * 4
                    bk = 0 if t0 < 196 else 1
                    r0 = (t0 - (0 if bk == 0 else 196)) * 128
                    for d in range(2):
                        dst = tabs[d][bk][r0:r0 + 512, :].rearrange(
                            "(t p) c -> p t c", t=4)
                        eng = nc.gpsimd if d == 0 else nc.scalar
                        eng.dma_start(dst, st[d][:])

                # local tiles (one per iteration)
                for t in range(nbin):
                    xt = pax.tile([128, 1, 2, 128], F16, tag="xt")
                    for k in range(2):
                        nc.sync.dma_start(
                            xt[:, 0, k, :],
                            xTl[k, :, t * 128:(t + 1) * 128])
                    for d in range(2):
                        st[d] = pas.tile([128, LW], F16, tag=f"lst{d}", name=f"lst{d}")
                    node_tile(xt[:, 0, :, :], 0, LW)
                    for d in range(2):
                        eng = nc.gpsimd if d == 0 else nc.scalar
                        eng.dma_start(locs[d][t * 128:(t + 1) * 128, :], st[d][:])

            # ---------------- Phase B ----------------
            with (
                tc.tile_pool(name="pBg", bufs=12) as pg,
                tc.tile_pool(name="pBk", bufs=8) as pk,
                tc.tile_pool(name="pBm", bufs=8) as pm,
                tc.tile_pool(name="pBo", bufs=2) as po,
                tc.tile_pool(name="psN", bufs=3, space="PSUM") as psn,
                tc.tile_pool(name="psE", bufs=2, space="PSUM") as pse,
            ):
                for b in range(nbin):
                    stage = [None, None]
                    for d in range(2):
                        gi = pm.tile([128, 16 * cb], I16, tag="gi")
                        nc.sync.dma_start(gi[:], gidx[d, b])
                        mk = pk.tile([128, 2, cpb * 128], F16, tag="mk")
                        nc.scalar.dma_start(mk[:], msk[d, b].rearrange("m p c -> p m c"))
                        lc = pm.tile([128, LW], F16, tag="lc")
                        nc.sync.dma_start(lc[:], locs[d][b * 128:(b + 1) * 128, :])

                        srcg = pg.tile([128, cpb, TW], F16, tag="srcg")
                        for half in range(2):
                            nc.gpsimd.dma_gather(
                                srcg[:, half * cb:(half + 1) * cb, :],
                                tabs[d][half][:],
                                gi[:, half * 8 * cb:(half + 1) * 8 * cb],
                                cb * 128, cb * 128, TW,
                                queue_num=(2 * (2 * b + d) + half) % 4,
                                single_packet=False)

                        if debug and b == 0 and d == 0:
                            nc.sync.dma_start(dbg_srcg[:], srcg[:].rearrange("p a b -> p (a b)"))
                            nc.sync.dma_start(dbg_lc[:], lc[:])
                        # ed_bc[e, h] via MT^T @ ed_tile
                        ps_ed = pse.tile([128, cpb, 4], F32, tag="ped")
                        for c in range(cpb):
                            nc.tensor.matmul(
                                ps_ed[:, c, :], mk[:, 1, c * 128:(c + 1) * 128],
                                lc[:, HC + 4:HC + 8], start=True, stop=True)
                        # logits l = es + ed_bc ; p = exp(lrelu(l) - ln64)
                        lg = pm.tile([128, cpb, 4], F32, tag="lg")
                        nc.vector.tensor_tensor(
                            lg[:], srcg[:, :, HC:HC + 4], ps_ed[:], OP.add)
                        lr = pm.tile([128, cpb, 4], F32, tag="lr")
                        nc.scalar.activation(lr[:], lg[:], AF.Prelu, alpha=SLOPE)
                        pf = pm.tile([128, cpb, 4], F32, tag="pf")
                        nc.scalar.activation(pf[:], lr[:], AF.Exp, bias=lnb_t[:])
                        p16 = srcg[:, :, HC + 4:HC + 8]
                        nc.scalar.activation(p16, pf[:], AF.Copy)

                        if debug and b == 0 and d == 0:
                            nc.sync.dma_start(dbg_p[:], p16[:].rearrange("p a b -> p (a b)"))
                            ped_sb = pm.tile([128, cpb, 4], F32, tag="pedsb")
                            nc.vector.tensor_copy(ped_sb[:], ps_ed[:])
                            nc.sync.dma_start(dbg_ped[:], ped_sb[:].rearrange("p a b -> p (a b)"))
                        # rows *= p (per-head broadcast multiply)
                        for c in range(cpb):
                            v = srcg[:, c, 0:HC].rearrange("p (h w) -> p h w", w=C)
                            nc.vector.tensor_tensor(
                                v, v,
                                srcg[:, c, HC + 4:HC + 8].unsqueeze(2)
                                .to_broadcast((128, H, C)),
                                OP.mult)

                        if debug and b == 0 and d == 0:
                            nc.sync.dma_start(dbg_srcg2[:], srcg[:].rearrange("p a b -> p (a b)"))
                        # num/den accumulation
                        pnd = psn.tile([128, HC + 8], F32, tag="pnd")
                        for c in range(cpb):
                            mc = mk[:, 0, c * 128:(c + 1) * 128]
                            nc.tensor.matmul(pnd[:, 0:HC + 8], mc,
                                             srcg[:, c, 0:HC + 8],
                                             start=(c == 0), stop=(c == cpb - 1))

                        if debug and b == 0 and d == 0:
                            pnd_sb = pm.tile([128, HC + 4], F32, tag="pndsb")
                            nc.vector.tensor_copy(pnd_sb[:, 0:HC], pnd[:, 0:HC])
                            nc.vector.tensor_copy(pnd_sb[:, HC:], pnd[:, HC + 4:HC + 8])
                            nc.sync.dma_start(dbg_pnd[:], pnd_sb[:])
                        # self-loop p
                        sl = pm.tile([128, 4], F32, tag="sl")
                        nc.vector.tensor_tensor(
                            sl[:], lc[:, HC:HC + 4], lc[:, HC + 4:HC + 8], OP.add)
                        slr = pm.tile([128, 4], F32, tag="slr")
                        nc.scalar.activation(slr[:], sl[:], AF.Prelu, alpha=SLOPE)
                        psf = pm.tile([128, 4], F32, tag="psf")
                        nc.scalar.activation(psf[:], slr[:], AF.Exp, bias=lnb_t[:])

                        # normalize: stage = (num + p_self*h_loc) / (2*(den+p_self))
                        dtot = pm.tile([128, 4], F32, tag="dtot")
                        nc.vector.tensor_tensor(dtot[:], pnd[:, HC + 4:HC + 8], psf[:],
                                                OP.add)
                        nc.vector.tensor_scalar(
                            out=dtot[:], in0=dtot[:], scalar1=2.0, scalar2=1e-30,
                            op0=OP.mult, op1=OP.max)
                        rec = pm.tile([128, 4], F32, tag="rec")
                        nc.vector.reciprocal(rec[:], dtot[:])

                        stg = po.tile([128, H, C], F32, tag=f"stg{d}", name=f"stg{d}")
                        for h in range(H):
                            nc.scalar.activation(
                                stg[:, h, :], lc[:, h * C:(h + 1) * C], AF.Copy,
                                scale=psf[:, h:h + 1])
                        nc.vector.tensor_tensor(
                            stg[:], stg[:],
                            pnd[:, 0:HC].rearrange("p (h w) -> p h w", w=C), OP.add)
                        for h in range(H):
                            nc.scalar.activation(
                                stg[:, h, :], stg[:, h, :], AF.Copy,
                                scale=rec[:, h:h + 1])
                        stage[d] = stg

                    ot = po.tile([128, HC], F32, tag="ot")
                    nc.vector.tensor_tensor(
                        ot[:].rearrange("p (h w) -> p h w", w=C),
                        stage[0][:], stage[1][:], OP.add)
                    nc.vector.tensor_tensor(ot[:], ot[:], bias_bc[:], OP.add)
                    nc.sync.dma_start(out[b * 128:(b + 1) * 128, :], ot[:])

    nc.compile()
    return nc


# ---------------------------------------------------------------- host side

def _wrap16(arr):
    """int idx array [n] -> dma_gather layout [128, n/16] int16 (replicated)."""
    n = len(arr)
    m = arr.reshape(n // 16, 16).astype(np.int16).T  # [16, n/16]
    return np.tile(m, (8, 1))


def prep_inputs(x, edge_index, W1, a_src1, a_dst1, b1, W2, a_src2, a_dst2, b2,
                nbin=NBIN, cb=CB):
    cpb = 2 * cb
    x = np.asarray(x, np.float32)
    ei = np.asarray(edge_index)
    src, dst = ei[0].astype(np.int64), ei[1].astype(np.int64)
    dirs = [(src, dst), (dst, src)]   # no self-loops; handled via local path

    # per-node degree by (dir, src-bank)
    deg = np.zeros((N, 4), np.int64)
    for j, (ss, dd) in enumerate(dirs):
        for bk in range(2):
            m = (ss >= B0REAL) == (bk == 1)
            deg[:, 2 * j + bk] = np.bincount(dd[m], minlength=N)

    # fp16 feature layouts
    x16 = x.astype(np.float16)
    xpad = np.zeros((NT * 128, DIN), np.float16)
    xpad[0:B0REAL] = x16[0:B0REAL]
    xpad[BKROWS:BKROWS + (N - B0REAL)] = x16[B0REAL:N]
    xTb = np.ascontiguousarray(xpad.T.reshape(2, 128, NT * 128))

    W_l = [np.asarray(W1, np.float32), np.asarray(W2, np.float32)]
    a_l = [(np.asarray(a_src1, np.float32), np.asarray(a_dst1, np.float32)),
           (np.asarray(a_src2, np.float32), np.asarray(a_dst2, np.float32))]
    cols = []
    for d in range(2):
        for a in a_l[d]:
            A = np.zeros((HC, H), np.float32)
            for h in range(H):
                A[h * C:(h + 1) * C, h] = a[h]
            cols.append(W_l[d] @ A)         # [256, 4]
    Wsb = np.zeros((2, 2, 128, HC + 8), np.float16)
    for d in range(2):
        wext = np.concatenate([W_l[d], cols[2 * d], cols[2 * d + 1]],
                              axis=1).astype(np.float16)  # [256, 264]
        for k in range(2):
            Wsb[d, k] = wext[k * 128:(k + 1) * 128, :]
    b_in = (0.5 * (np.asarray(b1) + np.asarray(b2))).astype(np.float32)
    b_in = b_in.reshape(1, HC)

    in_maps, perms = [], []
    for core in range(NCORES):
        lo = core * NPC
        nodes = np.arange(lo, lo + NPC)
        order = nodes[np.argsort(-deg[nodes].sum(1), kind="stable")]
        degs = deg[order]
        bins_load = np.zeros((nbin, 4), np.int64)
        bins_cnt = np.zeros(nbin, np.int64)
        node_blk = np.full(N, -1, np.int64)
        node_slot = np.full(N, -1, np.int64)
        for i_n in range(len(order)):
            dgl = degs[i_n]
            ok = (bins_cnt < 128) & ((bins_load + dgl) <= cb * 128).all(1)
            assert ok.any(), "bin packing failed; raise nbin/cb"
            cand = np.where(ok)[0]
            nl = (bins_load[cand] + dgl).max(1) * 1000 + bins_cnt[cand]
            i = cand[np.argmin(nl)]
            node_blk[order[i_n]] = i
            node_slot[order[i_n]] = bins_cnt[i]
            bins_load[i] += dgl
            bins_cnt[i] += 1

        perm = np.full(nbin * 128, -1, np.int64)
        perm[node_blk[nodes] * 128 + node_slot[nodes]] = nodes
        perms.append(perm)

        g_idx = np.zeros((2, nbin, 128, 16 * cb), np.int16)
        m_host = np.zeros((2, nbin, 2, 128, cpb * 128), np.float16)
        for d, (ss, dd) in enumerate(dirs):
            sel = (dd >= lo) & (dd < lo + NPC)
            es_, ed_ = ss[sel], dd[sel]
            blk = node_blk[ed_]
            bank = (es_ >= B0REAL).astype(np.int64)
            eo = np.lexsort((bank, blk))
            es_, ed_, blk, bank = es_[eo], ed_[eo], blk[eo], bank[eo]
            seg = blk * 2 + bank
            segbnd = np.flatnonzero(np.diff(seg, prepend=-1))
            within = np.arange(len(seg)) - np.repeat(segbnd, np.diff(
                np.append(segbnd, len(seg))))
            assert (within < cb * 128).all()
            slot = within + bank * (cb * 128)
            srcrel = np.where(bank == 0, es_, es_ - B0REAL)
            dslot = node_slot[ed_]
            s_idx = np.full((nbin, cpb * 128), PADIDX, np.int64)
            s_idx[blk, slot] = srcrel
            chunk, epart = slot // 128, slot % 128
            m_host[d, blk, 0, epart, chunk * 128 + dslot] = 1.0
            m_host[d, blk, 1, dslot, chunk * 128 + epart] = 1.0
            for bb in range(nbin):
                g_idx[d, bb, :, 0:8 * cb] = _wrap16(s_idx[bb, 0:cb * 128])
                g_idx[d, bb, :, 8 * cb:16 * cb] = _wrap16(s_idx[bb, cb * 128:])

        xloc = np.zeros((nbin * 128, DIN), np.float16)
        valid = perm >= 0
        xloc[valid] = x16[perm[valid]]
        xTl = np.ascontiguousarray(xloc.T.reshape(2, 128, nbin * 128))

        in_maps.append({
            "xTb": xTb, "xTl": xTl, "Wsb": Wsb, "b_in": b_in,
            "gidx": g_idx, "msk": m_host,
        })
    return in_maps, perms


_NC_CACHE = {}


def kernel(**inputs):
    last_err = None
    for nbin, cb in ((NBIN, CB), (53, 4), (51, 5), (55, 5)):
        try:
            in_maps, perms = prep_inputs(**inputs, nbin=nbin, cb=cb)
            break
        except AssertionError as e:
            last_err = e
    else:
        raise last_err
    key = (nbin, cb)
    if key not in _NC_CACHE:
        _NC_CACHE[key] = build_kernel(nbin, cb)
    nc = _NC_CACHE[key]
    res = run_bass_kernel_spmd(nc, in_maps, list(range(NCORES)))
    result = np.empty((N, HC), np.float32)
    for core in range(NCORES):
        o = res.results[core]["out"]
        p = perms[core]
        valid = p >= 0
        result[p[valid]] = o[valid]
    return result


# revision 24
# speedup vs baseline: 1.0112x; 1.0112x over previous
"""DirGATConv on 8 Trainium2 NeuronCores (Bass/Tile), v2.

Strategy (node/data parallel, no collectives):
  - Each core owns 6250 destination nodes, permuted into NBIN blocks of <=128
    by bin packing so every (block, direction, src-bank) has at most CB*128
    non-self-loop edges.
  - Phase A (replicated on every core): h = x @ W_d for all nodes plus the
    per-node attention projections es/ed = x @ (W_d a_*), written to two DRAM
    gather tables per direction (fp16 rows: 256 h | 4 es | 124 pad = 768 B;
    row count per bank <= 32767 because dma_gather indices are int16).  A
    bin-permuted local table per direction holds (h | es | ed) for the core's
    own destinations (544 B rows, read linearly in Phase B).
  - Phase B per (block, direction): dma_gather the source rows (one gather
    per src-bank), then with host-shipped 0/1 fp16 masks M [e,d] / MT [d,e]:
      ed_bc  = MT^T @ ed_tile                    (per-edge dst projection)
      p      = exp(lrelu(es + ed_bc) - ln 64)    (scalar engine; -ln64 keeps
                                                  h*p inside fp16 range)
      rows  *= p (per-head broadcast multiply), then one matmul per chunk
      num    = M^T @ rows, den = M^T @ p         (same stationary mask)
      out_d  = (num + p_self*h_loc) / (2*(den + p_self))
    Softmax normalization is exact because num and den are linear in p and
    any per-edge common factor (the -ln64 bias) cancels in num/den.
  - Host work is graph-structure-only (bin packing, gather indices, masks,
    layout transposes) plus standard weight fusion (W @ a projections).
"""

import numpy as np

import concourse.bacc as bacc
import concourse.mybir as mybir
import concourse.tile as tile
from concourse.bass_utils import run_bass_kernel_spmd
from concourse import library_config

# problem constants
N, E, DIN, H, C = 50000, 400000, 256, 4, 64
HC = H * C
ALPHA, SLOPE = 0.5, 0.2

# distribution constants
NCORES = 8
NPC = N // NCORES              # 6250 destinations per core
B0REAL = 24960                 # real nodes in bank 0 (nodes 0..24959)
BKROWS = 25088                 # rows per table bank (includes zero pad rows)
PADIDX = BKROWS - 1            # gather index for empty edge slots (zero row)
NT = 392                       # main node tiles (2 banks x 196)
NBIN = 50                      # destination blocks per core
CB = 4                         # gather chunks per (block, src-bank)
CPB = 2 * CB                   # chunks per block
NLOC = NBIN * 128
TW = 384                       # table row width (fp16) = 768 B
LW = 272                       # local row width (fp16) = 544 B
LNB = float(np.log(64.0))      # exp bias, cancels in num/den
F16 = mybir.dt.float16
F32 = mybir.dt.float32
I16 = mybir.dt.int16
AF = mybir.ActivationFunctionType
OP = mybir.AluOpType


def build_kernel(nbin=NBIN, cb=CB, debug=False):
    cpb = 2 * cb
    nc = bacc.Bacc("TRN2", num_swdge_queues=4)
    if debug:
        dbg_srcg = nc.dram_tensor("dbg_srcg", [128, cpb * TW], F16, kind="ExternalOutput")
        dbg_ped = nc.dram_tensor("dbg_ped", [128, cpb * 4], F32, kind="ExternalOutput")
        dbg_p = nc.dram_tensor("dbg_p", [128, cpb * 4], F16, kind="ExternalOutput")
        dbg_pnd = nc.dram_tensor("dbg_pnd", [128, HC + 4], F32, kind="ExternalOutput")
        dbg_lc = nc.dram_tensor("dbg_lc", [128, LW], F16, kind="ExternalOutput")
        dbg_srcg2 = nc.dram_tensor("dbg_srcg2", [128, cpb * TW], F16, kind="ExternalOutput")

    xTb = nc.dram_tensor("xTb", [2, 128, NT * 128], F16, kind="ExternalInput")
    xTl = nc.dram_tensor("xTl", [2, 128, nbin * 128], F16, kind="ExternalInput")
    Wsb = nc.dram_tensor("Wsb", [2, 2, 128, HC + 8], F16, kind="ExternalInput")
    b_in = nc.dram_tensor("b_in", [1, HC], F32, kind="ExternalInput")
    gidx = nc.dram_tensor("gidx", [2, nbin, 128, 16 * cb], I16, kind="ExternalInput")
    msk = nc.dram_tensor("msk", [2, nbin, 2, 128, cpb * 128], F16, kind="ExternalInput")
    out = nc.dram_tensor("out", [nbin * 128, HC], F32, kind="ExternalOutput")

    with tile.TileContext(nc) as tc:
        with (
            tc.tile_pool(name="dram", bufs=1, space="DRAM") as dpool,
            tc.tile_pool(name="const", bufs=1) as cpool,
        ):
            nc.gpsimd.load_library(library_config.mlp)

            tabs = [
                [dpool.tile([BKROWS, TW], F16, tag=f"tab{d}{k}", name=f"tab{d}{k}")
                 for k in range(2)]
                for d in range(2)
            ]
            locs = [dpool.tile([nbin * 128, LW], F16, tag=f"loc{d}", name=f"loc{d}")
                    for d in range(2)]

            # weights: w_sb[d] [128 din, 2 k, 264 = hc|es|ed]
            w_sb = [cpool.tile([128, 2, HC + 8], F16, tag=f"w{d}", name=f"w{d}")
                    for d in range(2)]
            for d in range(2):
                nc.sync.dma_start(
                    w_sb[d][:], Wsb[d].rearrange("k p c -> p k c"))
            bias_bc = cpool.tile([128, HC], F32)
            nc.sync.dma_start(bias_bc[:], b_in[:].to_broadcast((128, HC)))
            lnb_t = cpool.tile([128, 1], F32)
            nc.vector.memset(lnb_t[:], -LNB)

            # ---------------- Phase A ----------------
            with (
                tc.tile_pool(name="pAx", bufs=8) as pax,
                tc.tile_pool(name="pAs", bufs=6) as pas,
                tc.tile_pool(name="psA", bufs=4, space="PSUM") as psa,
            ):
                st = [None, None]

                def node_tile(xt_k, j, wide):
                    """One 128-node tile: xt_k [128, 2, 128]; write into
                    st[d][:, j, :] (wide=TW) or st[d] [128, LW] (wide=LW)."""
                    ph0 = psa.tile([128, HC + 8], F32, tag="ph0")
                    ph1 = psa.tile([128, HC + 8], F32, tag="ph1")
                    ph = [ph0, ph1]
                    for k in range(2):
                        for d in range(2):
                            nc.tensor.matmul(
                                ph[d][:], xt_k[:, k, :], w_sb[d][:, k, :],
                                start=(k == 0), stop=(k == 1))
                    if wide == TW:
                        nc.vector.tensor_copy(st[0][:, j, 0:HC + 8], ph[0][:])
                        nc.scalar.activation(st[1][:, j, 0:HC + 8], ph[1][:], AF.Copy)
                    else:
                        nc.vector.tensor_copy(st[0][:, 0:HC + 8], ph[0][:])
                        nc.scalar.activation(st[1][:, 0:HC + 8], ph[1][:], AF.Copy)

                # main tiles: 4-tile batches (bank boundary at tile 196 = 49*4)
                for it in range(NT // 4):
                    xt = pax.tile([128, 4, 2, 128], F16, tag="xt")
                    for k in range(2):
                        nc.sync.dma_start(
                            xt[:, :, k, :],
                            xTb[k, :, it * 512:(it + 1) * 512].rearrange(
                                "p (t c) -> p t c", c=128))
                    for d in range(2):
                        st[d] = pas.tile([128, 4, TW], F16, tag=f"st{d}", name=f"st{d}")
                    for t in range(4):
                        node_tile(xt[:, t, :, :], t, TW)
                    t0 = it * 4
                    bk = 0 if t0 < 196 else 1
                    r0 = (t0 - (0 if bk == 0 else 196)) * 128
                    for d in range(2):
                        dst = tabs[d][bk][r0:r0 + 512, :].rearrange(
                            "(t p) c -> p t c", t=4)
                        eng = nc.gpsimd if d == 0 else nc.scalar
                        eng.dma_start(dst, st[d][:])

                # local tiles (one per iteration)
                for t in range(nbin):
                    xt = pax.tile([128, 1, 2, 128], F16, tag="xt")
                    for k in range(2):
                        nc.sync.dma_start(
                            xt[:, 0, k, :],
                            xTl[k, :, t * 128:(t + 1) * 128])
                    for d in range(2):
                        st[d] = pas.tile([128, LW], F16, tag=f"lst{d}", name=f"lst{d}")
                    node_tile(xt[:, 0, :, :], 0, LW)
                    for d in range(2):
                        eng = nc.gpsimd if d == 0 else nc.scalar
                        eng.dma_start(locs[d][t * 128:(t + 1) * 128, :], st[d][:])

            # ---------------- Phase B ----------------
            with (
                tc.tile_pool(name="pBg", bufs=8) as pg,
                tc.tile_pool(name="pBk", bufs=6) as pk,
                tc.tile_pool(name="pBm", bufs=6) as pm,
                tc.tile_pool(name="pBo", bufs=2) as po,
                tc.tile_pool(name="psN", bufs=3, space="PSUM") as psn,
                tc.tile_pool(name="psE", bufs=2, space="PSUM") as pse,
            ):
                for b in range(nbin):
                    stage = [None, None]
                    for d in range(2):
                        gi = pm.tile([128, 16 * cb], I16, tag="gi")
                        nc.sync.dma_start(gi[:], gidx[d, b])
                        mk = pk.tile([128, 2, cpb * 128], F16, tag="mk")
                        nc.scalar.dma_start(mk[:], msk[d, b].rearrange("m p c -> p m c"))
                        lc = pm.tile([128, LW], F16, tag="lc")
                        nc.sync.dma_start(lc[:], locs[d][b * 128:(b + 1) * 128, :])

                        srcg = pg.tile([128, cpb, TW], F16, tag="srcg")
                        for half in range(2):
                            nc.gpsimd.dma_gather(
                                srcg[:, half * cb:(half + 1) * cb, :],
                                tabs[d][half][:],
                                gi[:, half * 8 * cb:(half + 1) * 8 * cb],
                                cb * 128, cb * 128, TW,
                                queue_num=(2 * (2 * b + d) + half) % 4,
                                single_packet=False)

                        if debug and b == 0 and d == 0:
                            nc.sync.dma_start(dbg_srcg[:], srcg[:].rearrange("p a b -> p (a b)"))
                            nc.sync.dma_start(dbg_lc[:], lc[:])
                        # ed_bc[e, h] via MT^T @ ed_tile
                        ps_ed = pse.tile([128, cpb, 4], F32, tag="ped")
                        for c in range(cpb):
                            nc.tensor.matmul(
                                ps_ed[:, c, :], mk[:, 1, c * 128:(c + 1) * 128],
                                lc[:, HC + 4:HC + 8], start=True, stop=True)
                        # logits l = es + ed_bc ; p = exp(lrelu(l) - ln64)
                        lg = pm.tile([128, cpb, 4], F32, tag="lg")
                        nc.vector.tensor_tensor(
                            lg[:], srcg[:, :, HC:HC + 4], ps_ed[:], OP.add)
                        lr = pm.tile([128, cpb, 4], F32, tag="lr")
                        nc.scalar.activation(lr[:], lg[:], AF.Prelu, alpha=SLOPE)
                        pf = pm.tile([128, cpb, 4], F32, tag="pf")
                        nc.scalar.activation(pf[:], lr[:], AF.Exp, bias=lnb_t[:])
                        p16 = srcg[:, :, HC + 4:HC + 8]
                        nc.scalar.activation(p16, pf[:], AF.Copy)

                        if debug and b == 0 and d == 0:
                            nc.sync.dma_start(dbg_p[:], p16[:].rearrange("p a b -> p (a b)"))
                            ped_sb = pm.tile([128, cpb, 4], F32, tag="pedsb")
                            nc.vector.tensor_copy(ped_sb[:], ps_ed[:])
                            nc.sync.dma_start(dbg_ped[:], ped_sb[:].rearrange("p a b -> p (a b)"))
                        # rows *= p (per-head broadcast multiply)
                        for c in range(cpb):
                            v = srcg[:, c, 0:HC].rearrange("p (h w) -> p h w", w=C)
                            nc.vector.tensor_tensor(
                                v, v,
                                srcg[:, c, HC + 4:HC + 8].unsqueeze(2)
                                .to_broadcast((128, H, C)),
                                OP.mult)

                        if debug and b == 0 and d == 0:
                            nc.sync.dma_start(dbg_srcg2[:], srcg[:].rearrange("p a b -> p (a b)"))
                        # num/den accumulation
                        pnd = psn.tile([128, HC + 8], F32, tag="pnd")
                        for c in range(cpb):
                            mc = mk[:, 0, c * 128:(c + 1) * 128]
                            nc.tensor.matmul(pnd[:, 0:HC + 8], mc,
                                             srcg[:, c, 0:HC + 8],
                                             start=(c == 0), stop=(c == cpb - 1))

                        if debug and b == 0 and d == 0:
                            pnd_sb = pm.tile([128, HC + 4], F32, tag="pndsb")
                            nc.vector.tensor_copy(pnd_sb[:, 0:HC], pnd[:, 0:HC])
                            nc.vector.tensor_copy(pnd_sb[:, HC:], pnd[:, HC + 4:HC + 8])
                            nc.sync.dma_start(dbg_pnd[:], pnd_sb[:])
                        # self-loop p
                        sl = pm.tile([128, 4], F32, tag="sl")
                        nc.vector.tensor_tensor(
                            sl[:], lc[:, HC:HC + 4], lc[:, HC + 4:HC + 8], OP.add)
                        slr = pm.tile([128, 4], F32, tag="slr")
                        nc.scalar.activation(slr[:], sl[:], AF.Prelu, alpha=SLOPE)
                        psf = pm.tile([128, 4], F32, tag="psf")
                        nc.scalar.activation(psf[:], slr[:], AF.Exp, bias=lnb_t[:])

                        # normalize: stage = (num + p_self*h_loc) / (2*(den+p_self))
                        dtot = pm.tile([128, 4], F32, tag="dtot")
                        nc.vector.tensor_tensor(dtot[:], pnd[:, HC + 4:HC + 8], psf[:],
                                                OP.add)
                        nc.vector.tensor_scalar(
                            out=dtot[:], in0=dtot[:], scalar1=2.0, scalar2=1e-30,
                            op0=OP.mult, op1=OP.max)
                        rec = pm.tile([128, 4], F32, tag="rec")
                        nc.vector.reciprocal(rec[:], dtot[:])

                        stg = po.tile([128, H, C], F32, tag=f"stg{d}", name=f"stg{d}")
                        for h in range(H):
                            nc.scalar.activation(
                                stg[:, h, :], lc[:, h * C:(h + 1) * C], AF.Copy,
                                scale=psf[:, h:h + 1])
                        nc.vector.tensor_tensor(
                            stg[:], stg[:],
                            pnd[:, 0:HC].rearrange("p (h w) -> p h w", w=C), OP.add)
                        for h in range(H):
                            nc.scalar.activation(
                                stg[:, h, :], stg[:, h, :], AF.Copy,
                                scale=rec[:, h:h + 1])
                        stage[d] = stg

                    ot = po.tile([128, HC], F32, tag="ot")
                    nc.vector.tensor_tensor(
                        ot[:].rearrange("p (h w) -> p h w", w=C),
                        stage[0][:], stage[1][:], OP.add)
                    nc.vector.tensor_tensor(ot[:], ot[:], bias_bc[:], OP.add)
                    nc.sync.dma_start(out[b * 128:(b + 1) * 128, :], ot[:])

    nc.compile()
    return nc


# ---------------------------------------------------------------- host side

def _wrap16(arr):
    """int idx array [n] -> dma_gather layout [128, n/16] int16 (replicated)."""
    n = len(arr)
    m = arr.reshape(n // 16, 16).astype(np.int16).T  # [16, n/16]
    return np.tile(m, (8, 1))


def prep_inputs(x, edge_index, W1, a_src1, a_dst1, b1, W2, a_src2, a_dst2, b2,
                nbin=NBIN, cb=CB):
    cpb = 2 * cb
    x = np.asarray(x, np.float32)
    ei = np.asarray(edge_index)
    src, dst = ei[0].astype(np.int64), ei[1].astype(np.int64)
    dirs = [(src, dst), (dst, src)]   # no self-loops; handled via local path

    # per-node degree by (dir, src-bank)
    deg = np.zeros((N, 4), np.int64)
    for j, (ss, dd) in enumerate(dirs):
        for bk in range(2):
            m = (ss >= B0REAL) == (bk == 1)
            deg[:, 2 * j + bk] = np.bincount(dd[m], minlength=N)

    # fp16 feature layouts
    x16 = x.astype(np.float16)
    xpad = np.zeros((NT * 128, DIN), np.float16)
    xpad[0:B0REAL] = x16[0:B0REAL]
    xpad[BKROWS:BKROWS + (N - B0REAL)] = x16[B0REAL:N]
    xTb = np.ascontiguousarray(xpad.T.reshape(2, 128, NT * 128))

    W_l = [np.asarray(W1, np.float32), np.asarray(W2, np.float32)]
    a_l = [(np.asarray(a_src1, np.float32), np.asarray(a_dst1, np.float32)),
           (np.asarray(a_src2, np.float32), np.asarray(a_dst2, np.float32))]
    cols = []
    for d in range(2):
        for a in a_l[d]:
            A = np.zeros((HC, H), np.float32)
            for h in range(H):
                A[h * C:(h + 1) * C, h] = a[h]
            cols.append(W_l[d] @ A)         # [256, 4]
    Wsb = np.zeros((2, 2, 128, HC + 8), np.float16)
    for d in range(2):
        wext = np.concatenate([W_l[d], cols[2 * d], cols[2 * d + 1]],
                              axis=1).astype(np.float16)  # [256, 264]
        for k in range(2):
            Wsb[d, k] = wext[k * 128:(k + 1) * 128, :]
    b_in = (0.5 * (np.asarray(b1) + np.asarray(b2))).astype(np.float32)
    b_in = b_in.reshape(1, HC)

    in_maps, perms = [], []
    for core in range(NCORES):
        lo = core * NPC
        nodes = np.arange(lo, lo + NPC)
        order = nodes[np.argsort(-deg[nodes].sum(1), kind="stable")]
        degs = deg[order]
        bins_load = np.zeros((nbin, 4), np.int64)
        bins_cnt = np.zeros(nbin, np.int64)
        node_blk = np.full(N, -1, np.int64)
        node_slot = np.full(N, -1, np.int64)
        for i_n in range(len(order)):
            dgl = degs[i_n]
            ok = (bins_cnt < 128) & ((bins_load + dgl) <= cb * 128).all(1)
            assert ok.any(), "bin packing failed; raise nbin/cb"
            cand = np.where(ok)[0]
            nl = (bins_load[cand] + dgl).max(1) * 1000 + bins_cnt[cand]
            i = cand[np.argmin(nl)]
            node_blk[order[i_n]] = i
            node_slot[order[i_n]] = bins_cnt[i]
            bins_load[i] += dgl
            bins_cnt[i] += 1

        perm = np.full(nbin * 128, -1, np.int64)
        perm[node_blk[nodes] * 128 + node_slot[nodes]] = nodes
        perms.append(perm)

        g_idx = np.zeros((2, nbin, 128, 16 * cb), np.int16)
        m_host = np.zeros((2, nbin, 2, 128, cpb * 128), np.float16)
        for d, (ss, dd) in enumerate(dirs):
            sel = (dd >= lo) & (dd < lo + NPC)
            es_, ed_ = ss[sel], dd[sel]
            blk = node_blk[ed_]
            bank = (es_ >= B0REAL).astype(np.int64)
            eo = np.lexsort((bank, blk))
            es_, ed_, blk, bank = es_[eo], ed_[eo], blk[eo], bank[eo]
            seg = blk * 2 + bank
            segbnd = np.flatnonzero(np.diff(seg, prepend=-1))
            within = np.arange(len(seg)) - np.repeat(segbnd, np.diff(
                np.append(segbnd, len(seg))))
            assert (within < cb * 128).all()
            slot = within + bank * (cb * 128)
            srcrel = np.where(bank == 0, es_, es_ - B0REAL)
            dslot = node_slot[ed_]
            s_idx = np.full((nbin, cpb * 128), PADIDX, np.int64)
            s_idx[blk, slot] = srcrel
            chunk, epart = slot // 128, slot % 128
            m_host[d, blk, 0, epart, chunk * 128 + dslot] = 1.0
            m_host[d, blk, 1, dslot, chunk * 128 + epart] = 1.0
            for bb in range(nbin):
                g_idx[d, bb, :, 0:8 * cb] = _wrap16(s_idx[bb, 0:cb * 128])
                g_idx[d, bb, :, 8 * cb:16 * cb] = _wrap16(s_idx[bb, cb * 128:])

        xloc = np.zeros((nbin * 128, DIN), np.float16)
        valid = perm >= 0
        xloc[valid] = x16[perm[valid]]
        xTl = np.ascontiguousarray(xloc.T.reshape(2, 128, nbin * 128))

        in_maps.append({
            "xTb": xTb, "xTl": xTl, "Wsb": Wsb, "b_in": b_in,
            "gidx": g_idx, "msk": m_host,
        })
    return in_maps, perms


_NC_CACHE = {}


def kernel(**inputs):
    last_err = None
    for nbin, cb in ((NBIN, CB), (53, 4), (51, 5), (55, 5)):
        try:
            in_maps, perms = prep_inputs(**inputs, nbin=nbin, cb=cb)
            break
        except AssertionError as e:
            last_err = e
    else:
        raise last_err
    key = (nbin, cb)
    if key not in _NC_CACHE:
        _NC_CACHE[key] = build_kernel(nbin, cb)
    nc = _NC_CACHE[key]
    res = run_bass_kernel_spmd(nc, in_maps, list(range(NCORES)))
    result = np.empty((N, HC), np.float32)
    for core in range(NCORES):
        o = res.results[core]["out"]
        p = perms[core]
        valid = p >= 0
        result[p[valid]] = o[valid]
    return result
